# revision 20
# baseline (speedup 1.0000x reference)
"""Trainium2 Bass kernel for nn_Actor (topk_masking) — final.

Reference semantics:
    s    = einsum('ol,bld->bod', W, state)[:, 0, :]        # (B, D) sum over L
    a0   = softmax(s, axis=-1)
    loop T-1 times: zero the argmax entry, renormalize; stack all T states
    out  = (B, T, D)

Identity: out[t] = (e < v_t) * e * C_t with e = exp(w * sum_l x_l), v_t the
t-th largest of e, C_t = 1/D_t, D_t = suffix sum of top-24 values + R.

Design (measured ~250 us, rel err 2e-4, vs 403 us fp32 PE+DVE baseline):
  - Split-precision input stream: host splits each f32 element into
    hi = f16(x) plus f16 residuals, with residuals of 4 adjacent l's
    pre-summed (12 quads + 1 pair, scaled by 1024 and fed through a
    (1/1024)*I stationary).  63 f16 slices per (tile, d-half) = 3 bytes
    per input element -> 64.5 MB/core of loads instead of 102.4.
  - The L-sum runs on TensorE: f16 identity matmuls accumulate one PSUM
    bank per (tile, half) at ~1 cycle/row (fp32 matmul is 4).  The last
    6 lo slices are summed on DVE in f32 instead, which keeps the PE ~9%
    ahead of the DMA stream per chunk — at exact PE/DMA parity, HAM
    cold-start cascades cost ~25-50 us of run-to-run variance.
    Precision note: top-k masking needs the s-ordering to match the
    reference almost exactly — f32r (or f16-only input) gives s errors
    ~1e-4 that flip near-equal top values and corrupt whole rows
    (2.7e-2 rel err); the hi+lo split keeps s error ~5e-6.
  - exp() reads PSUM directly; all 8 PSUM banks cycle so the PE never
    waits on the ACT queue to free a bank.
  - Stats: 3x max8 + 2x match_replace give the top-24 values; rows 8 and
    16 reuse the match_replace outputs (exact top-8/16-zeroed tensors);
    R comes from (sum beyond top-16) - sum(vc) — no third match_replace.
  - Rows: per row one DVE scalar_tensor_tensor mask (f32, tie-safe) and
    one ACT scale-copy to f16.  fp16/bf16 intermediates for the rows do
    not help: STT has no fast 16-bit mode, the f16->f16 4x ACT mode is
    fragile in situ, f16 masks underflow on peaked rows, and f16-domain
    compares collide on near-equal top values.
  - Loads ride the sync-engine HWDGE ring, stores the ACT ring, so the
    load stream never blocks behind epilogue stores (the v1 kernel lost
    ~5 us per tile boundary + a 29 us tail to this).  The last tile
    streams rows out individually to shrink the exposed tail.
  - Output rows staged f16 (tolerance 2e-2 >> f16 ~1e-4), widened on
    host.

Sharding: pure data parallel over the batch dim across 8 NeuronCores.
"""

import numpy as np

from concourse import bacc, bass, mybir
from concourse import tile as tile_mod
from concourse.bass_utils import run_bass_kernel_spmd

F32 = mybir.dt.float32
F32R = mybir.dt.float32r
F16 = mybir.dt.float16
BF16 = mybir.dt.bfloat16
AF = mybir.ActivationFunctionType
ALU = mybir.AluOpType

# Problem constants (hardcoded per harness contract)
B_FULL = 4096
L = 50
D = 1000
T = 20
N_CORES = 8
B_CORE = B_FULL // N_CORES  # 512
P = 128                     # partitions per tile
N_TILES = B_CORE // P       # 4
DH = D // 2                 # 500 = one PSUM bank of f32

L_LO = 7                    # lo residuals: 6 octets (8 l's) + 1 pair
N_SLICE = L + L_LO          # 57 f16 slices per (tile, half): 7 lo + 50 hi
N_DVE = 6                   # lo slices summed on DVE (exact f32 path)
CHUNK_LS = (12, 12, 11, 11, 11)   # slices per load chunk
N_CHUNK = len(CHUNK_LS)
LO_SCALE = 1024.0           # lo parts pre-scaled by 1024 (keeps f16 normal)
ROWS_PER_STORE = 5          # output rows per store DMA (1.25 MB fp16)


def build_graph(w_scale=1.0):
    """Uniform-weight path: all 50 l's summed on PE via f32r identity
    accumulation; the uniform weight folds into the exp scale."""
    nc = bacc.Bacc("TRN2")
    # state pre-split on host into d-halves and f16 hi/lo parts:
    # hi = f16(x), lo = f16(1024*(x - hi)); each [b, L, 500] contiguous
    # unified stream per d-half: slices 0..6 = f16 lo residual octets
    # (first, so the DVE consumes them early), slices 7..56 = f16 hi
    half_ext = [
        nc.declare_dram_parameter(f"state{h}", [B_CORE, N_SLICE, DH], F16,
                                  isOutput=False)
        for h in range(2)
    ]
    # wmat[:, 0, :] = I (hi pass), wmat[:, 1, :] = I/1024 (lo pass)
    wmat_ext = nc.declare_dram_parameter("wmat", [P, 2, P], F16,
                                         isOutput=False)
    out_ext = nc.declare_dram_parameter("out", [B_CORE, T, D], F16,
                                        isOutput=True)

    with tile_mod.TileContext(nc) as tc:
        with (
            tc.tile_pool(name="chunk", bufs=10) as chunk_pool,
            tc.tile_pool(name="part", bufs=2) as part_pool,
            tc.tile_pool(name="epool", bufs=3) as e_pool,
            tc.tile_pool(name="tmp", bufs=6) as tmp_pool,
            tc.tile_pool(name="rows", bufs=3) as row_pool,
            tc.tile_pool(name="small", bufs=2) as small_pool,
            tc.tile_pool(name="wm", bufs=1) as w_pool,
            tc.tile_pool(name="ps", bufs=7, space="PSUM") as ps_pool,
            tc.tile_pool(name="warmps", bufs=1, space="PSUM") as warm_pool,
        ):
            wm = w_pool.tile([P, 2, P], F16, tag="wm")
            nc.sync.dma_start(wm[:], wmat_ext[:])

            # warm-up burst: ~3.4us of PE activity flips the HAM clock
            # gate to full rate before the first real group arrives
            warm = warm_pool.tile([P, P], F32, tag="warm")
            for i in range(24):
                nc.tensor.matmul(warm[:], wm[:, 0, :], wm[:, 0, :],
                                 start=True, stop=True)

            def emit_rows(bt, b0, e0, v_pad, Ct, u8, u16):
                # rows: t=0 plain; t=8/16 fall out of the match_replace
                # chain; the rest are threshold-masked.  Stores ride the
                # ACT ring so the sync-engine load stream never waits on
                # epilogue compute; the last tile streams each row out
                # individually to shrink the exposed tail.
                rps = ROWS_PER_STORE
                rowgs = {}
                for t in range(T):
                    g = t // rps
                    j = t % rps
                    if g not in rowgs:
                        rowgs[g] = row_pool.tile(
                            [P, rps, D], F16, tag="rowg",
                            name=f"rowg_{bt}_{g}",
                        )
                    if t == 0:
                        src_row = e0
                    elif t == 8:
                        src_row = u8     # e0 with exactly top-8 zeroed
                    elif t == 16:
                        src_row = u16    # e0 with exactly top-16 zeroed
                    else:
                        src_row = tmp_pool.tile([P, D], F32, tag="tmp")
                        # (e0 < v_t) * e0 ; v_t = t-th largest = v_pad[6+t]
                        nc.vector.scalar_tensor_tensor(
                            src_row[:], e0[:], v_pad[:, 6 + t : 7 + t],
                            e0[:], ALU.is_lt, ALU.mult,
                        )
                    nc.scalar.activation(
                        rowgs[g][:, j, :], src_row[:], AF.Copy, bias=0.0,
                        scale=Ct[:, t : t + 1],
                    )
                    if bt == N_TILES - 1:
                        # tail stores ride the sync engine: it has no
                        # loads left, and store triggers on the ACT queue
                        # (~600ns each) would starve the DVE row chain
                        nc.sync.dma_start(
                            out_ext[b0 : b0 + P, t : t + 1, :],
                            rowgs[g][:, j : j + 1, :],
                        )
                    elif j == rps - 1:
                        nc.scalar.dma_start(
                            out_ext[b0 : b0 + P, t - j : t + 1, :],
                            rowgs[g][:],
                        )

            for bt in range(N_TILES):
                b0 = bt * P
                e0 = e_pool.tile([P, D], F32, tag="e")

                for h in range(2):
                    d0 = h * DH
                    pt = ps_pool.tile([P, DH], F32, tag="p")
                    g0 = 0
                    ch0 = None
                    for m in range(N_CHUNK):
                        cl = CHUNK_LS[m]
                        ch = chunk_pool.tile([P, CHUNK_LS[0], DH], F16,
                                             tag="ch")
                        if m == 0:
                            ch0 = ch
                        nc.sync.dma_start(
                            ch[:, 0:cl, :],
                            half_ext[h][b0 : b0 + P, g0 : g0 + cl, :],
                        )
                        # DVE takes lo slices 0..N_DVE-1 from the first
                        # chunk (keeps PE ~10% ahead of the DMA stream:
                        # at parity, HAM cold-start cascades cost ~25us
                        # of run-to-run variance); PE takes the rest
                        for j in range(cl):
                            g = g0 + j
                            if g >= N_DVE:
                                nc.tensor.matmul(
                                    pt[:],
                                    wm[:, 1 if g < L_LO else 0, :],
                                    ch[:, j, :],
                                    start=(g == N_DVE),
                                    stop=(g == N_SLICE - 1),
                                )
                        g0 += cl
                    # DVE: sum the 6 lo octet slices in f32 (exact),
                    # scale by 1/1024 and add the PSUM part
                    lw = part_pool.tile([P, DH], F32, tag="part")
                    nc.vector.tensor_tensor(
                        lw[:], ch0[:, 0, :], ch0[:, 1, :], ALU.add,
                    )
                    for j in range(2, N_DVE):
                        nc.vector.tensor_tensor(
                            lw[:], lw[:], ch0[:, j, :], ALU.add
                        )
                    nc.vector.scalar_tensor_tensor(
                        lw[:], lw[:], 1.0 / 1024.0, pt[:], ALU.mult,
                        ALU.add,
                    )
                    # e-half: exp (uniform w folded into the scale)
                    nc.scalar.activation(
                        e0[:, d0 : d0 + DH], lw[:], AF.Exp, bias=0.0,
                        scale=float(w_scale),
                    )

                # ---- top-24 values + R = sum of the rest ----
                st = small_pool.tile([P, 104], F32, tag="stats")
                v_pad = st[:, 0:31]
                suf = st[:, 32:56]
                Dt = st[:, 56:76]
                Ct = st[:, 76:96]
                R = st[:, 96:97]
                nc.vector.memset(v_pad[:, 0:7], -1.0)
                va = v_pad[:, 7:15]
                vb = v_pad[:, 15:23]
                vc = v_pad[:, 23:31]
                u8 = tmp_pool.tile([P, D], F32, tag="tmp")
                u16 = tmp_pool.tile([P, D], F32, tag="tmp")
                nc.vector.max(va, e0[:])
                nc.vector.match_replace(u8[:], va, e0[:], 0.0)
                nc.vector.max(vb, u8[:])
                nc.vector.match_replace(u16[:], vb, u8[:], 0.0)
                nc.vector.max(vc, u16[:])
                # R = (sum beyond top-16) - sum(vc): saves the third
                # match_replace pass; no cancellation risk since vc are
                # the largest components of the top-16-masked residual
                R16 = st[:, 97:98]
                vcs = st[:, 98:99]
                nc.vector.tensor_reduce(
                    R16, u16[:], axis=mybir.AxisListType.X, op=ALU.add
                )
                nc.vector.tensor_reduce(
                    vcs, vc, axis=mybir.AxisListType.X, op=ALU.add
                )
                nc.vector.tensor_tensor(R, R16[:], vcs[:], ALU.subtract)

                # ---- D_t = suffix_sum(v_{t+1..24}) + R ;  C = 1/D ----
                v_rev = v_pad[:, 30:6:-1]
                nc.vector.tensor_tensor_scan(
                    suf, v_rev, v_rev, 0.0, ALU.add, ALU.bypass
                )
                nc.vector.tensor_scalar(
                    Dt, suf[:, 23:3:-1], R, None, ALU.add
                )
                nc.vector.reciprocal(Ct, Dt)
                emit_rows(bt, b0, e0, v_pad, Ct, u8, u16)

    nc.finalize()
    return nc


# ---------------------------------------------------------------------------
# General (non-uniform weight) fallback: per-l diag(w_l) fp32 stationaries.
# Unused by the harness (weight_matrix is all-ones) but kept for correctness.
def build_graph_general():
    MEGA_L = 5
    N_MEGA = L // MEGA_L
    nc = bacc.Bacc("TRN2")
    half_ext = [
        nc.declare_dram_parameter(f"state{h}", [B_CORE, L, DH], F32,
                                  isOutput=False)
        for h in range(2)
    ]
    wmat_ext = nc.declare_dram_parameter("wmat", [P, L, P], F32,
                                         isOutput=False)
    out_ext = nc.declare_dram_parameter("out", [B_CORE, T, D], F16,
                                        isOutput=True)

    with tile_mod.TileContext(nc) as tc:
        with (
            tc.tile_pool(name="mega", bufs=13) as mega_pool,
            tc.tile_pool(name="epool", bufs=3) as e_pool,
            tc.tile_pool(name="tmp", bufs=6) as tmp_pool,
            tc.tile_pool(name="rows", bufs=3) as row_pool,
            tc.tile_pool(name="small", bufs=2) as small_pool,
            tc.tile_pool(name="wm", bufs=1) as w_pool,
            tc.tile_pool(name="ps", bufs=7, space="PSUM") as ps_pool,
            tc.tile_pool(name="warmps", bufs=1, space="PSUM") as warm_pool,
        ):
            wm = w_pool.tile([P, L, P], F32, tag="wm")
            nc.sync.dma_start(wm[:], wmat_ext[:])

            for bt in range(N_TILES):
                b0 = bt * P
                e0 = e_pool.tile([P, D], F32, tag="e")
                for h in range(2):
                    d0 = h * DH
                    pt = ps_pool.tile([P, DH], F32, tag="p")
                    for m in range(N_MEGA):
                        M = mega_pool.tile([P, MEGA_L, DH], F32, tag="mega")
                        nc.sync.dma_start(
                            M[:],
                            half_ext[h][
                                b0 : b0 + P,
                                m * MEGA_L : (m + 1) * MEGA_L,
                                :,
                            ],
                        )
                        for j in range(MEGA_L):
                            l = m * MEGA_L + j
                            nc.tensor.matmul(
                                pt[:], wm[:, l, :], M[:, j, :],
                                start=(l == 0), stop=(l == L - 1),
                            )
                    nc.scalar.activation(
                        e0[:, d0 : d0 + DH], pt[:], AF.Exp, bias=0.0,
                        scale=1.0,
                    )

                st = small_pool.tile([P, 104], F32, tag="stats")
                v_pad = st[:, 0:31]
                suf = st[:, 32:56]
                Dt = st[:, 56:76]
                Ct = st[:, 76:96]
                R = st[:, 96:97]
                nc.vector.memset(v_pad[:, 0:7], -1.0)
                va = v_pad[:, 7:15]
                vb = v_pad[:, 15:23]
                vc = v_pad[:, 23:31]
                u8 = tmp_pool.tile([P, D], F32, tag="tmp")
                u16 = tmp_pool.tile([P, D], F32, tag="tmp")
                nc.vector.max(va, e0[:])
                nc.vector.match_replace(u8[:], va, e0[:], 0.0)
                nc.vector.max(vb, u8[:])
                nc.vector.match_replace(u16[:], vb, u8[:], 0.0)
                nc.vector.max(vc, u16[:])
                nc.vector.match_replace(u[:], vc, u[:], 0.0)
                nc.vector.tensor_reduce(
                    R, u[:], axis=mybir.AxisListType.X, op=ALU.add
                )
                v_rev = v_pad[:, 30:6:-1]
                nc.vector.tensor_tensor_scan(
                    suf, v_rev, v_rev, 0.0, ALU.add, ALU.bypass
                )
                nc.vector.tensor_scalar(
                    Dt, suf[:, 23:3:-1], R, None, ALU.add
                )
                nc.vector.reciprocal(Ct, Dt)

                rowgs = {}
                for t in range(T):
                    g = t // ROWS_PER_STORE
                    j = t % ROWS_PER_STORE
                    if g not in rowgs:
                        rowgs[g] = row_pool.tile(
                            [P, ROWS_PER_STORE, D], F16, tag="rowg",
                            name=f"rowg_{bt}_{g}",
                        )
                    if t == 0:
                        src_row = e0
                    else:
                        src_row = tmp_pool.tile([P, D], F32, tag="tmp")
                        nc.vector.scalar_tensor_tensor(
                            src_row[:], e0[:], v_pad[:, 6 + t : 7 + t],
                            e0[:], ALU.is_lt, ALU.mult,
                        )
                    nc.scalar.activation(
                        rowgs[g][:, j, :], src_row[:], AF.Copy, bias=0.0,
                        scale=Ct[:, t : t + 1],
                    )
                    if j == ROWS_PER_STORE - 1:
                        nc.scalar.dma_start(
                            out_ext[b0 : b0 + P, t - j : t + 1, :],
                            rowgs[g][:],
                        )

    nc.finalize()
    return nc


_GRAPH_CACHE = {}


def _get_graph(w):
    w = np.asarray(w, dtype=np.float32).reshape(-1)
    assert w.shape[0] == L
    if np.all(w == w[0]):
        key = ("uniform", float(w[0]))
        if key not in _GRAPH_CACHE:
            _GRAPH_CACHE[key] = build_graph(w_scale=float(w[0]))
        wmat = np.zeros((P, 2, P), dtype=np.float16)
        np.fill_diagonal(wmat[:, 0, :], np.float16(1.0))
        np.fill_diagonal(wmat[:, 1, :], np.float16(1.0 / LO_SCALE))
        return _GRAPH_CACHE[key], wmat, False
    key = "general"
    if key not in _GRAPH_CACHE:
        _GRAPH_CACHE[key] = build_graph_general()
    wmat = np.zeros((P, L, P), dtype=np.float32)
    for l in range(L):
        np.fill_diagonal(wmat[:, l, :], w[l])
    return _GRAPH_CACHE[key], wmat, True


def kernel(state, weight_matrix):
    state = np.ascontiguousarray(np.asarray(state, dtype=np.float32))
    w = np.asarray(weight_matrix, dtype=np.float32)
    assert state.shape == (B_FULL, L, D), state.shape

    nc, in_maps = _prepare(state, w)
    res = run_bass_kernel_spmd(nc, in_maps, core_ids=list(range(N_CORES)))
    out = np.concatenate(
        [
            np.asarray(res.results[i]["out"], dtype=np.float32)
            for i in range(N_CORES)
        ],
        axis=0,
    )
    return out


def _prepare(state, w):
    nc, wmat, general = _get_graph(w)
    if general:
        d_lo = np.ascontiguousarray(state[:, :, :DH])
        d_hi = np.ascontiguousarray(state[:, :, DH:])
        in_maps = [
            {
                "state0": d_lo[i * B_CORE : (i + 1) * B_CORE],
                "state1": d_hi[i * B_CORE : (i + 1) * B_CORE],
                "wmat": wmat,
            }
            for i in range(N_CORES)
        ]
    else:
        # split each element into f16 hi + scaled f16 lo residual; lo
        # residuals of adjacent l-pairs are pre-summed on host (halves the
        # lo traffic; error ~2^-20 abs, small enough to keep the top-k
        # ordering aligned with the reference)
        hi = state.astype(np.float16)
        r = state - hi.astype(np.float32)
        B = r.shape[0]
        octs = r[:, 0:48, :].reshape(B, 6, 8, D).sum(axis=2)
        pair = r[:, 48:50, :].sum(axis=1, keepdims=True)
        lo = (np.concatenate([octs, pair], axis=1) * LO_SCALE).astype(
            np.float16)
        uni = np.concatenate([lo, hi], axis=1)  # (B, 57, 1000) f16
        in_maps = [
            {
                "state0": np.ascontiguousarray(
                    uni[i * B_CORE : (i + 1) * B_CORE, :, :DH]),
                "state1": np.ascontiguousarray(
                    uni[i * B_CORE : (i + 1) * B_CORE, :, DH:]),
                "wmat": wmat,
            }
            for i in range(N_CORES)
        ]
    return nc, in_maps


# revision 21
# speedup vs baseline: 1.1490x; 1.1490x over previous
"""Trainium2 Bass kernel for nn_Actor (topk_masking) — final.

Reference semantics:
    s    = einsum('ol,bld->bod', W, state)[:, 0, :]        # (B, D) sum over L
    a0   = softmax(s, axis=-1)
    loop T-1 times: zero the argmax entry, renormalize; stack all T states
    out  = (B, T, D)

Identity: out[t] = (e < v_t) * e * C_t with e = exp(w * sum_l x_l), v_t the
t-th largest of e, C_t = 1/D_t, D_t = suffix sum of top-24 values + R.

Design (measured ~250 us, rel err 2e-4, vs 403 us fp32 PE+DVE baseline):
  - Split-precision input stream: host splits each f32 element into
    hi = f16(x) plus f16 residuals, with residuals of 4 adjacent l's
    pre-summed (12 quads + 1 pair, scaled by 1024 and fed through a
    (1/1024)*I stationary).  63 f16 slices per (tile, d-half) = 3 bytes
    per input element -> 64.5 MB/core of loads instead of 102.4.
  - The L-sum runs on TensorE: f16 identity matmuls accumulate one PSUM
    bank per (tile, half) at ~1 cycle/row (fp32 matmul is 4).  The last
    6 lo slices are summed on DVE in f32 instead, which keeps the PE ~9%
    ahead of the DMA stream per chunk — at exact PE/DMA parity, HAM
    cold-start cascades cost ~25-50 us of run-to-run variance.
    Precision note: top-k masking needs the s-ordering to match the
    reference almost exactly — f32r (or f16-only input) gives s errors
    ~1e-4 that flip near-equal top values and corrupt whole rows
    (2.7e-2 rel err); the hi+lo split keeps s error ~5e-6.
  - exp() reads PSUM directly; all 8 PSUM banks cycle so the PE never
    waits on the ACT queue to free a bank.
  - Stats: 3x max8 + 2x match_replace give the top-24 values; rows 8 and
    16 reuse the match_replace outputs (exact top-8/16-zeroed tensors);
    R comes from (sum beyond top-16) - sum(vc) — no third match_replace.
  - Rows: per row one DVE scalar_tensor_tensor mask (f32, tie-safe) and
    one ACT scale-copy to f16.  fp16/bf16 intermediates for the rows do
    not help: STT has no fast 16-bit mode, the f16->f16 4x ACT mode is
    fragile in situ, f16 masks underflow on peaked rows, and f16-domain
    compares collide on near-equal top values.
  - Loads ride the sync-engine HWDGE ring, stores the ACT ring, so the
    load stream never blocks behind epilogue stores (the v1 kernel lost
    ~5 us per tile boundary + a 29 us tail to this).  The last tile
    streams rows out individually to shrink the exposed tail.
  - Output rows staged f16 (tolerance 2e-2 >> f16 ~1e-4), widened on
    host.

Sharding: pure data parallel over the batch dim across 8 NeuronCores.
"""

import numpy as np

from concourse import bacc, bass, mybir
from concourse import tile as tile_mod
from concourse.bass_utils import run_bass_kernel_spmd

F32 = mybir.dt.float32
F32R = mybir.dt.float32r
F16 = mybir.dt.float16
BF16 = mybir.dt.bfloat16
AF = mybir.ActivationFunctionType
ALU = mybir.AluOpType

# Problem constants (hardcoded per harness contract)
B_FULL = 4096
L = 50
D = 1000
T = 20
N_CORES = 8
B_CORE = B_FULL // N_CORES  # 512
P = 128                     # partitions per tile
N_TILES = B_CORE // P       # 4
DH = D // 2                 # 500 = one PSUM bank of f32

L_LO = 7                    # lo residuals: 6 octets (8 l's) + 1 pair
N_SLICE = L + L_LO          # 57 f16 slices per (tile, half): 7 lo + 50 hi
N_DVE = 6                   # lo slices summed on DVE (exact f32 path)
CHUNK_LS = (12, 12, 11, 11, 11)   # slices per load chunk
N_CHUNK = len(CHUNK_LS)
LO_SCALE = 1024.0           # lo parts pre-scaled by 1024 (keeps f16 normal)
ROWS_PER_STORE = 5          # output rows per store DMA (1.25 MB fp16)


def build_graph(w_scale=1.0):
    """Uniform-weight path: all 50 l's summed on PE via f32r identity
    accumulation; the uniform weight folds into the exp scale."""
    nc = bacc.Bacc("TRN2")
    # state pre-split on host into d-halves and f16 hi/lo parts:
    # hi = f16(x), lo = f16(1024*(x - hi)); each [b, L, 500] contiguous
    # unified stream per d-half: slices 0..6 = f16 lo residual octets
    # (first, so the DVE consumes them early), slices 7..56 = f16 hi
    half_ext = [
        nc.declare_dram_parameter(f"state{h}", [B_CORE, N_SLICE, DH], F16,
                                  isOutput=False)
        for h in range(2)
    ]
    # wmat[:, 0, :] = I (hi pass), wmat[:, 1, :] = I/1024 (lo pass)
    wmat_ext = nc.declare_dram_parameter("wmat", [P, 2, P], F16,
                                         isOutput=False)
    out_ext = nc.declare_dram_parameter("out", [B_CORE, T, D], F16,
                                        isOutput=True)

    with tile_mod.TileContext(nc) as tc:
        with (
            tc.tile_pool(name="chunk", bufs=10) as chunk_pool,
            tc.tile_pool(name="part", bufs=2) as part_pool,
            tc.tile_pool(name="epool", bufs=3) as e_pool,
            tc.tile_pool(name="tmp", bufs=6) as tmp_pool,
            tc.tile_pool(name="rows", bufs=3) as row_pool,
            tc.tile_pool(name="small", bufs=2) as small_pool,
            tc.tile_pool(name="wm", bufs=1) as w_pool,
            tc.tile_pool(name="ps", bufs=7, space="PSUM") as ps_pool,
            tc.tile_pool(name="warmps", bufs=1, space="PSUM") as warm_pool,
        ):
            wm = w_pool.tile([P, 2, P], F16, tag="wm")
            nc.sync.dma_start(wm[:], wmat_ext[:])

            # warm-up burst: ~3.4us of PE activity flips the HAM clock
            # gate to full rate before the first real group arrives
            warm = warm_pool.tile([P, P], F32, tag="warm")
            for i in range(24):
                nc.tensor.matmul(warm[:], wm[:, 0, :], wm[:, 0, :],
                                 start=True, stop=True)

            def emit_rows(bt, b0, e0, v_pad, Ct, u8, u16):
                # rows: t=0 plain; t=8/16 fall out of the match_replace
                # chain; the rest are threshold-masked.  Stores ride the
                # ACT ring so the sync-engine load stream never waits on
                # epilogue compute; the last tile streams each row out
                # individually to shrink the exposed tail.
                rps = ROWS_PER_STORE
                rowgs = {}
                for t in range(T):
                    g = t // rps
                    j = t % rps
                    if g not in rowgs:
                        rowgs[g] = row_pool.tile(
                            [P, rps, D], F16, tag="rowg",
                            name=f"rowg_{bt}_{g}",
                        )
                    if t == 0:
                        src_row = e0
                    elif t == 8:
                        src_row = u8     # e0 with exactly top-8 zeroed
                    elif t == 16:
                        src_row = u16    # e0 with exactly top-16 zeroed
                    else:
                        src_row = tmp_pool.tile([P, D], F32, tag="tmp")
                        # (e0 < v_t) * e0 ; v_t = t-th largest = v_pad[6+t]
                        nc.vector.scalar_tensor_tensor(
                            src_row[:], e0[:], v_pad[:, 6 + t : 7 + t],
                            e0[:], ALU.is_lt, ALU.mult,
                        )
                    nc.scalar.activation(
                        rowgs[g][:, j, :], src_row[:], AF.Copy, bias=0.0,
                        scale=Ct[:, t : t + 1],
                    )
                    if bt == N_TILES - 1:
                        nc.scalar.dma_start(
                            out_ext[b0 : b0 + P, t : t + 1, :],
                            rowgs[g][:, j : j + 1, :],
                        )
                    elif j == rps - 1:
                        nc.scalar.dma_start(
                            out_ext[b0 : b0 + P, t - j : t + 1, :],
                            rowgs[g][:],
                        )

            for bt in range(N_TILES):
                b0 = bt * P
                e0 = e_pool.tile([P, D], F32, tag="e")

                for h in range(2):
                    d0 = h * DH
                    pt = ps_pool.tile([P, DH], F32, tag="p")
                    g0 = 0
                    ch0 = None
                    for m in range(N_CHUNK):
                        cl = CHUNK_LS[m]
                        ch = chunk_pool.tile([P, CHUNK_LS[0], DH], F16,
                                             tag="ch")
                        if m == 0:
                            ch0 = ch
                        nc.sync.dma_start(
                            ch[:, 0:cl, :],
                            half_ext[h][b0 : b0 + P, g0 : g0 + cl, :],
                        )
                        # DVE takes lo slices 0..N_DVE-1 from the first
                        # chunk (keeps PE ~10% ahead of the DMA stream:
                        # at parity, HAM cold-start cascades cost ~25us
                        # of run-to-run variance); PE takes the rest
                        for j in range(cl):
                            g = g0 + j
                            if g >= N_DVE:
                                nc.tensor.matmul(
                                    pt[:],
                                    wm[:, 1 if g < L_LO else 0, :],
                                    ch[:, j, :],
                                    start=(g == N_DVE),
                                    stop=(g == N_SLICE - 1),
                                )
                        g0 += cl
                    # DVE: sum the 6 lo octet slices in f32 (exact),
                    # scale by 1/1024 and add the PSUM part
                    lw = part_pool.tile([P, DH], F32, tag="part")
                    nc.vector.tensor_tensor(
                        lw[:], ch0[:, 0, :], ch0[:, 1, :], ALU.add,
                    )
                    for j in range(2, N_DVE):
                        nc.vector.tensor_tensor(
                            lw[:], lw[:], ch0[:, j, :], ALU.add
                        )
                    nc.vector.scalar_tensor_tensor(
                        lw[:], lw[:], 1.0 / 1024.0, pt[:], ALU.mult,
                        ALU.add,
                    )
                    # e-half: exp (uniform w folded into the scale)
                    nc.scalar.activation(
                        e0[:, d0 : d0 + DH], lw[:], AF.Exp, bias=0.0,
                        scale=float(w_scale),
                    )

                # ---- top-24 values + R = sum of the rest ----
                st = small_pool.tile([P, 104], F32, tag="stats")
                v_pad = st[:, 0:31]
                suf = st[:, 32:56]
                Dt = st[:, 56:76]
                Ct = st[:, 76:96]
                R = st[:, 96:97]
                nc.vector.memset(v_pad[:, 0:7], -1.0)
                va = v_pad[:, 7:15]
                vb = v_pad[:, 15:23]
                vc = v_pad[:, 23:31]
                u8 = tmp_pool.tile([P, D], F32, tag="tmp")
                u16 = tmp_pool.tile([P, D], F32, tag="tmp")
                nc.vector.max(va, e0[:])
                nc.vector.match_replace(u8[:], va, e0[:], 0.0)
                nc.vector.max(vb, u8[:])
                nc.vector.match_replace(u16[:], vb, u8[:], 0.0)
                nc.vector.max(vc, u16[:])
                # R = (sum beyond top-16) - sum(vc): saves the third
                # match_replace pass; no cancellation risk since vc are
                # the largest components of the top-16-masked residual
                R16 = st[:, 97:98]
                vcs = st[:, 98:99]
                nc.vector.tensor_reduce(
                    R16, u16[:], axis=mybir.AxisListType.X, op=ALU.add
                )
                nc.vector.tensor_reduce(
                    vcs, vc, axis=mybir.AxisListType.X, op=ALU.add
                )
                nc.vector.tensor_tensor(R, R16[:], vcs[:], ALU.subtract)

                # ---- D_t = suffix_sum(v_{t+1..24}) + R ;  C = 1/D ----
                v_rev = v_pad[:, 30:6:-1]
                nc.vector.tensor_tensor_scan(
                    suf, v_rev, v_rev, 0.0, ALU.add, ALU.bypass
                )
                nc.vector.tensor_scalar(
                    Dt, suf[:, 23:3:-1], R, None, ALU.add
                )
                nc.vector.reciprocal(Ct, Dt)
                emit_rows(bt, b0, e0, v_pad, Ct, u8, u16)

    nc.finalize()
    return nc


# ---------------------------------------------------------------------------
# General (non-uniform weight) fallback: per-l diag(w_l) fp32 stationaries.
# Unused by the harness (weight_matrix is all-ones) but kept for correctness.
def build_graph_general():
    MEGA_L = 5
    N_MEGA = L // MEGA_L
    nc = bacc.Bacc("TRN2")
    half_ext = [
        nc.declare_dram_parameter(f"state{h}", [B_CORE, L, DH], F32,
                                  isOutput=False)
        for h in range(2)
    ]
    wmat_ext = nc.declare_dram_parameter("wmat", [P, L, P], F32,
                                         isOutput=False)
    out_ext = nc.declare_dram_parameter("out", [B_CORE, T, D], F16,
                                        isOutput=True)

    with tile_mod.TileContext(nc) as tc:
        with (
            tc.tile_pool(name="mega", bufs=13) as mega_pool,
            tc.tile_pool(name="epool", bufs=3) as e_pool,
            tc.tile_pool(name="tmp", bufs=6) as tmp_pool,
            tc.tile_pool(name="rows", bufs=3) as row_pool,
            tc.tile_pool(name="small", bufs=2) as small_pool,
            tc.tile_pool(name="wm", bufs=1) as w_pool,
            tc.tile_pool(name="ps", bufs=7, space="PSUM") as ps_pool,
            tc.tile_pool(name="warmps", bufs=1, space="PSUM") as warm_pool,
        ):
            wm = w_pool.tile([P, L, P], F32, tag="wm")
            nc.sync.dma_start(wm[:], wmat_ext[:])

            for bt in range(N_TILES):
                b0 = bt * P
                e0 = e_pool.tile([P, D], F32, tag="e")
                for h in range(2):
                    d0 = h * DH
                    pt = ps_pool.tile([P, DH], F32, tag="p")
                    for m in range(N_MEGA):
                        M = mega_pool.tile([P, MEGA_L, DH], F32, tag="mega")
                        nc.sync.dma_start(
                            M[:],
                            half_ext[h][
                                b0 : b0 + P,
                                m * MEGA_L : (m + 1) * MEGA_L,
                                :,
                            ],
                        )
                        for j in range(MEGA_L):
                            l = m * MEGA_L + j
                            nc.tensor.matmul(
                                pt[:], wm[:, l, :], M[:, j, :],
                                start=(l == 0), stop=(l == L - 1),
                            )
                    nc.scalar.activation(
                        e0[:, d0 : d0 + DH], pt[:], AF.Exp, bias=0.0,
                        scale=1.0,
                    )

                st = small_pool.tile([P, 104], F32, tag="stats")
                v_pad = st[:, 0:31]
                suf = st[:, 32:56]
                Dt = st[:, 56:76]
                Ct = st[:, 76:96]
                R = st[:, 96:97]
                nc.vector.memset(v_pad[:, 0:7], -1.0)
                va = v_pad[:, 7:15]
                vb = v_pad[:, 15:23]
                vc = v_pad[:, 23:31]
                u8 = tmp_pool.tile([P, D], F32, tag="tmp")
                u16 = tmp_pool.tile([P, D], F32, tag="tmp")
                nc.vector.max(va, e0[:])
                nc.vector.match_replace(u8[:], va, e0[:], 0.0)
                nc.vector.max(vb, u8[:])
                nc.vector.match_replace(u16[:], vb, u8[:], 0.0)
                nc.vector.max(vc, u16[:])
                nc.vector.match_replace(u[:], vc, u[:], 0.0)
                nc.vector.tensor_reduce(
                    R, u[:], axis=mybir.AxisListType.X, op=ALU.add
                )
                v_rev = v_pad[:, 30:6:-1]
                nc.vector.tensor_tensor_scan(
                    suf, v_rev, v_rev, 0.0, ALU.add, ALU.bypass
                )
                nc.vector.tensor_scalar(
                    Dt, suf[:, 23:3:-1], R, None, ALU.add
                )
                nc.vector.reciprocal(Ct, Dt)

                rowgs = {}
                for t in range(T):
                    g = t // ROWS_PER_STORE
                    j = t % ROWS_PER_STORE
                    if g not in rowgs:
                        rowgs[g] = row_pool.tile(
                            [P, ROWS_PER_STORE, D], F16, tag="rowg",
                            name=f"rowg_{bt}_{g}",
                        )
                    if t == 0:
                        src_row = e0
                    else:
                        src_row = tmp_pool.tile([P, D], F32, tag="tmp")
                        nc.vector.scalar_tensor_tensor(
                            src_row[:], e0[:], v_pad[:, 6 + t : 7 + t],
                            e0[:], ALU.is_lt, ALU.mult,
                        )
                    nc.scalar.activation(
                        rowgs[g][:, j, :], src_row[:], AF.Copy, bias=0.0,
                        scale=Ct[:, t : t + 1],
                    )
                    if j == ROWS_PER_STORE - 1:
                        nc.scalar.dma_start(
                            out_ext[b0 : b0 + P, t - j : t + 1, :],
                            rowgs[g][:],
                        )

    nc.finalize()
    return nc


_GRAPH_CACHE = {}


def _get_graph(w):
    w = np.asarray(w, dtype=np.float32).reshape(-1)
    assert w.shape[0] == L
    if np.all(w == w[0]):
        key = ("uniform", float(w[0]))
        if key not in _GRAPH_CACHE:
            _GRAPH_CACHE[key] = build_graph(w_scale=float(w[0]))
        wmat = np.zeros((P, 2, P), dtype=np.float16)
        np.fill_diagonal(wmat[:, 0, :], np.float16(1.0))
        np.fill_diagonal(wmat[:, 1, :], np.float16(1.0 / LO_SCALE))
        return _GRAPH_CACHE[key], wmat, False
    key = "general"
    if key not in _GRAPH_CACHE:
        _GRAPH_CACHE[key] = build_graph_general()
    wmat = np.zeros((P, L, P), dtype=np.float32)
    for l in range(L):
        np.fill_diagonal(wmat[:, l, :], w[l])
    return _GRAPH_CACHE[key], wmat, True


def kernel(state, weight_matrix):
    state = np.ascontiguousarray(np.asarray(state, dtype=np.float32))
    w = np.asarray(weight_matrix, dtype=np.float32)
    assert state.shape == (B_FULL, L, D), state.shape

    nc, in_maps = _prepare(state, w)
    res = run_bass_kernel_spmd(nc, in_maps, core_ids=list(range(N_CORES)))
    out = np.concatenate(
        [
            np.asarray(res.results[i]["out"], dtype=np.float32)
            for i in range(N_CORES)
        ],
        axis=0,
    )
    return out


def _prepare(state, w):
    nc, wmat, general = _get_graph(w)
    if general:
        d_lo = np.ascontiguousarray(state[:, :, :DH])
        d_hi = np.ascontiguousarray(state[:, :, DH:])
        in_maps = [
            {
                "state0": d_lo[i * B_CORE : (i + 1) * B_CORE],
                "state1": d_hi[i * B_CORE : (i + 1) * B_CORE],
                "wmat": wmat,
            }
            for i in range(N_CORES)
        ]
    else:
        # split each element into f16 hi + scaled f16 lo residual; lo
        # residuals of adjacent l-pairs are pre-summed on host (halves the
        # lo traffic; error ~2^-20 abs, small enough to keep the top-k
        # ordering aligned with the reference)
        hi = state.astype(np.float16)
        r = state - hi.astype(np.float32)
        B = r.shape[0]
        octs = r[:, 0:48, :].reshape(B, 6, 8, D).sum(axis=2)
        pair = r[:, 48:50, :].sum(axis=1, keepdims=True)
        lo = (np.concatenate([octs, pair], axis=1) * LO_SCALE).astype(
            np.float16)
        uni = np.concatenate([lo, hi], axis=1)  # (B, 57, 1000) f16
        in_maps = [
            {
                "state0": np.ascontiguousarray(
                    uni[i * B_CORE : (i + 1) * B_CORE, :, :DH]),
                "state1": np.ascontiguousarray(
                    uni[i * B_CORE : (i + 1) * B_CORE, :, DH:]),
                "wmat": wmat,
            }
            for i in range(N_CORES)
        ]
    return nc, in_maps
